# revision 18
# baseline (speedup 1.0000x reference)
"""ACSF descriptor kernel for 8 Trainium2 NeuronCores (Bass/Tile), v4.

Scheme
------
* Output atoms sharded across 8 cores (6250 each).  Host does integer
  topology routing plus standard neighbor-list cutoff pruning: G4
  triplets with D_ba >= rc, D_ca >= rc or R_bc >= rc contribute exactly
  zero (the reference multiplies them by 0) and are dropped (37%); G2
  edges with D > 5.8 contribute < 3e-3 of one output unit (fc(5.8) =
  2.7e-3) and are dropped (tolerance is 2e-2; measured error ~1e-3).
* Atoms are packed BLK per block (G4: 10, G2: 7) so each block is
  exactly ONE 128-item tile; ONE matmul per block scatters 15 (G4
  "moment" planes av_i cos^k) / 8 (G2) values into [W, 3*BLK] PSUM via
  a tiny fp8 one-hot.  The PE retires (LDWEIGHTS, MATMUL) pairs at a
  fixed ~34ns regardless of width, so minimizing matmul count (= block
  count) is everything: 628 + 896 MMs per core.
* Host reconstructs the (lambda, zeta) G4 columns from the moments with
  binomial combinations (exact), and scales by 2^(1-z)/8 and 0.5.
* No activation-table thrash: 3 input cosines (Sin) run up front, then
  Exp/Square only (2 table loads).  fc(R_bc) is a degree-5 polynomial
  in R^2/36 (pinned to 0 at rc) on DVE; dba/dca are host-clamped to 6.
  3 of 8 G2 exps are ACT squares of others (eta ratios are powers of 2).
* fp16 streams/intermediates/outputs; values chunked (geometrically
  growing chunks) so the matmul stream starts early; PSUM->SBUF copies
  alternate between Scalar and Vector and are emitted one chunk late to
  avoid head-of-line blocking.
"""

import math
from contextlib import ExitStack

import numpy as np

P = 128
N_ATOMS = 50000
N_CORES = 8
APC = N_ATOMS // N_CORES          # 6250 atoms per core
BLK4, W4 = 10, 15                 # G4: atoms/block, value planes
M4 = 3 * BLK4
BLK2, W2 = 7, 8                   # G2
M2 = 2 * BLK2
SQ = 16                           # quads per supertile (64 blocks)
RC = 6.0
G2_CUT = 5.8                      # G2 prune radius (fc(5.8) = 2.7e-3)


def _fit_poly():
    y = np.linspace(0.0, 1.0, 4001)
    V = np.vander(y, 6, increasing=True)
    c, *_ = np.linalg.lstsq(V, 1.0 + np.cos(np.pi * np.sqrt(y)), rcond=None)
    c[0] -= c.sum()
    return c.astype(np.float64)

POLY = _fit_poly()


# --------------------------------------------------------------------------
# host-side planning (integer topology work + cutoff pruning only)
# --------------------------------------------------------------------------

def _lpt_pack(cnts, nb, blkcap):
    """Place atoms (desc by count) into the least-loaded block that still
    has atom and item capacity.  Returns None if nb blocks don't suffice."""
    import heapq
    order = np.argsort(-cnts, kind="stable")
    heap = [(0, 0, b) for b in range(nb)]
    heapq.heapify(heap)
    items = np.zeros(nb, np.int64)
    nat = np.zeros(nb, np.int64)
    blk = np.empty(APC, np.int32)
    alo = np.empty(APC, np.int32)
    for a in order:
        c = int(cnts[a])
        stash = []
        ok = False
        while heap:
            it, na, b = heapq.heappop(heap)
            if it != items[b] or na != nat[b]:
                continue
            if na >= blkcap or it + c > P:
                stash.append((it, na, b))
                continue
            blk[a] = b
            alo[a] = na
            items[b] += c
            nat[b] += 1
            if nat[b] < blkcap:
                heapq.heappush(heap, (items[b], nat[b], b))
            ok = True
            break
        for s in stash:
            heapq.heappush(heap, s)
        if not ok:
            return None
    return blk, alo


def _balance(cnt_all, blkcap):
    nb = -(-max(-(-APC // blkcap), int(-(-cnt_all.sum(1).max() // P))) // 4) * 4
    while True:
        packs = []
        for c in range(N_CORES):
            r = _lpt_pack(cnt_all[c], nb, blkcap)
            if r is None:
                break
            packs.append(r)
        if len(packs) == N_CORES:
            return (np.stack([p[0] for p in packs]),
                    np.stack([p[1] for p in packs]), nb)
        nb += 8


def _pack(n_global, slot, feats, defaults, blk_of, aloc_of, nblk, M, nsl):
    """Route items into (core, block, partition) slots; emit fp16 value
    streams [C, 128, nblk] and the fp8 one-hot [C, 128, nblk*M]."""
    core = n_global // APC
    al = n_global % APC
    b = blk_of[core, al].astype(np.int64)
    nv = aloc_of[core, al].astype(np.int64) * nsl + slot
    key = core * nblk + b
    cnt = np.bincount(key, minlength=N_CORES * nblk)
    assert cnt.max() <= P, cnt.max()
    order = np.argsort(key, kind="stable")
    starts = np.zeros(N_CORES * nblk, np.int64)
    np.cumsum(cnt[:-1], out=starts[1:])
    rank = np.arange(len(key), dtype=np.int64) - np.repeat(starts, cnt)
    ko = key[order]
    co, bo, po = ko // nblk, ko % nblk, rank
    arrs = []
    for f, d in zip(feats, defaults):
        a = np.full((N_CORES, P, nblk), d, np.float16)
        a[co, po, bo] = f[order].astype(np.float16)
        arrs.append(a)
    oh = np.zeros((N_CORES, P, nblk * M), np.uint8)
    oh[co, po, bo * M + nv[order]] = 0x38          # 1.0 in fp8e4m3
    return arrs, oh


def _plan(inputs):
    an = np.asarray(inputs["atomic_numbers"])
    ei = np.asarray(inputs["edge_index"])
    D_st = np.asarray(inputs["D_st"], np.float32)
    ba = np.asarray(inputs["id3_ba"])
    ca = np.asarray(inputs["id3_ca"])
    cph = np.asarray(inputs["cos_phi"], np.float32)
    imap = np.asarray(inputs["idx_mapping"])
    imap2 = np.asarray(inputs["idx_mapping_g2"])
    src, dst = ei[0], ei[1]

    # ---- G4: integer mask + cutoff prune + destination/slot computation
    keep = ba > ca
    ba = ba[keep]; ca = ca[keep]; ch = cph[keep]
    db = D_st[ba]; dc = D_st[ca]
    alive = (db < RC) & (dc < RC) & (db * db + dc * dc - 2 * db * dc * ch
                                     < RC * RC)
    ba = ba[alive]; ca = ca[alive]
    n4 = dst[ca]
    p4 = imap[an[dst[ca]], an[src[ba]], an[src[ca]]]
    cnt4 = np.bincount(n4, minlength=N_ATOMS).reshape(N_CORES, APC)
    blk4, aloc4, nb4 = _balance(cnt4, BLK4)
    g4_arrs, oh4 = _pack(
        n4, p4, [db[alive], dc[alive], ch[alive]],
        [1.0, 1.0, 0.0], blk4, aloc4, nb4, M4, 3)

    # ---- G2: prune negligible-contribution edges
    alive2 = D_st <= G2_CUT
    n2 = dst[alive2]
    s2 = imap2[an[dst[alive2]], an[src[alive2]]]
    cnt2 = np.bincount(n2, minlength=N_ATOMS).reshape(N_CORES, APC)
    blk2, aloc2, nb2 = _balance(cnt2, BLK2)
    g2_arrs, oh2 = _pack(n2, s2, [D_st[alive2]], [1.0], blk2, aloc2,
                         nb2, M2, 2)

    g2_etas = np.asarray(inputs["G2_params"], np.float64)[0, 0]        # [8]
    etas = np.asarray(inputs["G4_etas"], np.float64)[0, 0, 0]          # [3]
    zetas = np.asarray(inputs["G4_zetas"], np.float64)[0, 0, 0]        # [3]
    lmdas = np.asarray(inputs["G4_lmdas"], np.float64)[0, 0, 0]        # [2]
    assert np.allclose(zetas, [1.0, 2.0, 4.0]), zetas
    assert np.allclose(sorted(lmdas), [-1.0, 1.0]), lmdas
    # eta relations used to replace 3 of the 8 G2 exps with ACT squarings
    assert abs(g2_etas[2] - 2 * g2_etas[1]) < 1e-4 * g2_etas[2]
    assert abs(g2_etas[6] - 2 * g2_etas[5]) < 1e-4 * g2_etas[6]
    assert abs(g2_etas[7] - 2 * g2_etas[6]) < 1e-4 * g2_etas[7]
    assert etas[0] * 220.0 < 0.1        # linearized exp(-eta0*s) stays accurate

    return dict(
        dba=g4_arrs[0], dca=g4_arrs[1], cph=g4_arrs[2], oh4=oh4,
        d2=g2_arrs[0], oh2=oh2,
        blk4=blk4, aloc4=aloc4, nb4=nb4,
        blk2=blk2, aloc2=aloc2, nb2=nb2,
        g2_etas=g2_etas, etas=etas, zetas=zetas, lmdas=lmdas,
    )


def _combo_matrix(zetas, lmdas):
    B = np.zeros((6, 5), np.float64)
    for li, l in enumerate(lmdas):
        for zi, z in enumerate(zetas):
            cz = 0.125 * 2.0 ** (1.0 - z)
            for k in range(int(z) + 1):
                B[li * 3 + zi, k] = math.comb(int(z), k) * (l ** k) * cz
    return B


def _assemble(o4s, o2s, plan):
    nb4, nb2 = plan["nb4"], plan["nb2"]
    B = _combo_matrix(plan["zetas"], plan["lmdas"])
    full = np.empty((N_ATOMS, 70), np.float32)
    for c in range(N_CORES):
        V4 = np.asarray(o4s[c], np.float32).reshape(4, 32, nb4 // 4, BLK4, 3)
        A4 = V4[:, :W4].transpose(2, 0, 3, 1, 4).reshape(nb4 * BLK4, W4, 3)
        r4 = plan["blk4"][c].astype(np.int64) * BLK4 + plan["aloc4"][c]
        Mm = A4[r4].reshape(APC, 3, 5, 3)
        g4 = np.einsum('aiks,wk->aiws', Mm, B).reshape(APC, 54)
        V2 = np.asarray(o2s[c], np.float32).reshape(4, 32, nb2 // 4, BLK2, 2)
        A2 = V2[:, :W2].transpose(2, 0, 3, 1, 4).reshape(nb2 * BLK2, W2, 2)
        r2 = plan["blk2"][c].astype(np.int64) * BLK2 + plan["aloc2"][c]
        g2 = (A2[r2] * 0.5).reshape(APC, 16)
        full[c * APC:(c + 1) * APC, :16] = g2
        full[c * APC:(c + 1) * APC, 16:] = g4
    return full


def _chunks(nb):
    """Geometrically growing chunk plan aligned to supertiles (64 blocks)."""
    out = []
    t0 = 0
    for w in (64, 96, 128, 192, 256, 256, 256, 256):
        if t0 >= nb:
            break
        cw = min(w, nb - t0)
        out.append((t0, cw))
        t0 += cw
    while t0 < nb:
        out.append((t0, min(256, nb - t0)))
        t0 += min(256, nb - t0)
    return out


# --------------------------------------------------------------------------
# numpy simulation of the device program (for host-side validation)
# --------------------------------------------------------------------------

def _simulate_core(plan, c):
    f16 = np.float16
    e1c, e2c, e3c = plan["etas"]
    g2e = plan["g2_etas"]
    dba = plan["dba"][c]; dca = plan["dca"][c]; cph = plan["cph"][c]
    d2 = plan["d2"][c]
    co = POLY

    def act(x, f):
        return f(x.astype(np.float32)).astype(f16)

    b2 = act(dba, np.square); c2 = act(dca, np.square)
    c2n = act(cph, np.square)
    bc = (dba * dca).astype(f16)
    c3 = (c2n * cph).astype(f16)
    c4 = (c2n * c2n).astype(f16)
    t4 = (b2 + c2).astype(f16)
    bcc = (bc * cph).astype(f16)
    u = (t4 - bcc).astype(f16)                      # s = 2u
    r2 = (u - bcc).astype(f16)
    yc = np.minimum((r2 * f16(1 / 36.)).astype(f16), f16(1.0))
    v = (yc * f16(co[5])).astype(f16)
    for k in (4, 3, 2, 1):
        v = ((v + f16(co[k])) * yc).astype(f16)
    ub = act(dba, lambda x: np.sin(np.pi / 2 - np.pi / 6 * x))
    uc = act(dca, lambda x: np.sin(np.pi / 2 - np.pi / 6 * x))
    uu = ((ub + f16(1.0)) * (uc + f16(1.0)).astype(f16)).astype(f16)
    cut = ((v + f16(co[0])) * uu).astype(f16)
    e1 = (u * f16(-2 * e1c) + f16(1.0)).astype(f16)
    e2 = act(u, lambda x: np.exp(-2 * e2c * x))
    e3 = act(u, lambda x: np.exp(-2 * e3c * x))
    pows = [None, cph, c2n, c3, c4]
    v15 = np.empty((P, W4, plan["nb4"]), f16)
    for i, e in enumerate((e1, e2, e3)):
        av = (e * cut).astype(f16)
        v15[:, i * 5] = av
        for k in range(1, 5):
            v15[:, i * 5 + k] = (av * pows[k]).astype(f16)
    oh4 = (plan["oh4"][c] == 0x38).reshape(P, plan["nb4"], M4)
    o4 = np.zeros((P, plan["nb4"] // 4 * M4), np.float32)
    for b in range(plan["nb4"]):
        q, g = b // 4, b % 4
        o4[32 * g:32 * g + W4, q * M4:(q + 1) * M4] = (
            v15[:, :, b].astype(np.float32).T @ oh4[:, b].astype(np.float32))
    q2 = act(d2, np.square)
    h = act(d2, lambda x: np.sin(np.pi / 2 - np.pi / 6 * x))
    hp = (h + f16(1.0)).astype(f16)
    ge = {}
    for k in (0, 1, 3, 4, 5):
        ge[k] = act(q2, lambda x, kk=k: np.exp(-g2e[kk] * x))
    ge[2] = act(ge[1], np.square)
    ge[6] = act(ge[5], np.square)
    ge[7] = act(ge[6], np.square)
    v8 = np.empty((P, W2, plan["nb2"]), f16)
    for k in range(8):
        v8[:, k] = (hp * ge[k]).astype(f16)
    oh2 = (plan["oh2"][c] == 0x38).reshape(P, plan["nb2"], M2)
    o2 = np.zeros((P, plan["nb2"] // 4 * M2), np.float32)
    for b in range(plan["nb2"]):
        q, g = b // 4, b % 4
        o2[32 * g:32 * g + W2, q * M2:(q + 1) * M2] = (
            v8[:, :, b].astype(np.float32).T @ oh2[:, b].astype(np.float32))
    return o4.astype(f16), o2.astype(f16)


def simulate(inputs):
    plan = _plan(inputs)
    outs = [_simulate_core(plan, c) for c in range(N_CORES)]
    return _assemble([o[0] for o in outs], [o[1] for o in outs], plan)


# --------------------------------------------------------------------------
# Bass/Tile device kernel
# --------------------------------------------------------------------------

def _build_nc(nb4, nb2, consts):
    import concourse.bacc as bacc
    import concourse.tile as tile
    from concourse import mybir

    f32 = mybir.dt.float32
    f16 = mybir.dt.float16
    f8 = mybir.dt.float8e4
    AF = mybir.ActivationFunctionType
    OP = mybir.AluOpType
    etas, g2e = consts["etas"], consts["g2_etas"]
    co = [float(x) for x in POLY]

    nc = bacc.Bacc(None, target_bir_lowering=False)
    dba_d = nc.dram_tensor("dba", [P, nb4], f16, kind="ExternalInput")
    dca_d = nc.dram_tensor("dca", [P, nb4], f16, kind="ExternalInput")
    cph_d = nc.dram_tensor("cph", [P, nb4], f16, kind="ExternalInput")
    d2_d = nc.dram_tensor("d2", [P, nb2], f16, kind="ExternalInput")
    oh4_d = nc.dram_tensor("oh4", [P, nb4 * M4], f8, kind="ExternalInput")
    oh2_d = nc.dram_tensor("oh2", [P, nb2 * M2], f8, kind="ExternalInput")
    out4_d = nc.dram_tensor("out4", [P, nb4 // 4 * M4], f16,
                            kind="ExternalOutput")
    out2_d = nc.dram_tensor("out2", [P, nb2 // 4 * M2], f16,
                            kind="ExternalOutput")

    ch4 = _chunks(nb4)
    ch2 = _chunks(nb2)
    cwmax = 256

    with tile.TileContext(nc) as tc, ExitStack() as ctx:
        inp = ctx.enter_context(tc.tile_pool(name="inp", bufs=1))
        scr = ctx.enter_context(tc.tile_pool(name="scr", bufs=3))
        vchk = ctx.enter_context(tc.tile_pool(name="vchk", bufs=3))
        ohp = ctx.enter_context(tc.tile_pool(name="ohp", bufs=4))
        outp = ctx.enter_context(tc.tile_pool(name="outp", bufs=3))
        psp = ctx.enter_context(tc.tile_pool(name="psum", bufs=4, space="PSUM"))

        V, A, G = nc.vector, nc.scalar, nc.gpsimd

        consts_sb = {}

        def const(v):
            v = float(v)
            if v not in consts_sb:
                tl = inp.tile([P, 1], f32, tag="const%r" % v,
                              name="c%d" % len(consts_sb))
                nc.vector.memset(tl[:], v)
                consts_sb[v] = tl[:]
            return consts_sb[v]

        oh_tiles = {}

        def oh_fetch(fam, ci):
            key = (fam, ci)
            if key not in oh_tiles:
                M, nb, dd, cl = ((M2, nb2, oh2_d, ch2) if fam == "2"
                                 else (M4, nb4, oh4_d, ch4))
                t0, cw = cl[ci]
                t = ohp.tile([P, cwmax * M], f8, tag="oh" + fam,
                             name="oh%s_%d" % (fam, ci))
                nc.sync.dma_start(out=t[:, :cw * M],
                                  in_=dd[:, t0 * M:(t0 + cw) * M])
                oh_tiles[key] = t
            return oh_tiles[key]

        # dummy activations: hoist the two ACT table loads ahead of the
        # input DMAs (table loads have no data dependency)
        dum = inp.tile([P, 1], f16, tag="dum", name="dum")
        A.activation(out=dum[:], in_=const(0.0), func=AF.Sin)

        # ---- input DMAs: G2's dependencies first so its MMs start early ----
        sb = {}
        sb["d2"] = inp.tile([P, nb2], f16, tag="d2", name="sb_d2")
        nc.sync.dma_start(out=sb["d2"][:], in_=d2_d[:])
        oh_fetch("2", 0)
        for nm, dd in (("dba", dba_d), ("dca", dca_d), ("cph", cph_d)):
            sb[nm] = inp.tile([P, nb4], f16, tag=nm, name="sb_" + nm)
            nc.sync.dma_start(out=sb[nm][:], in_=dd[:])
        oh_fetch("4", 0)
        dba, dca, cph, d2 = (sb[k][:] for k in ("dba", "dca", "cph", "d2"))

        # ---- the only Sin-table users, then Exp/Square forever ----
        h = inp.tile([P, nb2], f16, tag="h", name="h")
        ub = inp.tile([P, nb4], f16, tag="ub", name="ub")
        ucp = inp.tile([P, nb4], f16, tag="ucp", name="ucp")
        uu = inp.tile([P, nb4], f16, tag="uu", name="uu")
        with tc.high_priority():
            A.activation(out=h[:], in_=d2, func=AF.Sin,
                         bias=const(math.pi / 2), scale=const(-math.pi / 6))
        def uu_emit():
            V.tensor_scalar(out=ucp[:], in0=ucp[:], scalar1=1.0, scalar2=None,
                            op0=OP.add)
            V.scalar_tensor_tensor(out=uu[:], in0=ub[:], scalar=1.0,
                                   in1=ucp[:], op0=OP.add, op1=OP.mult)

        hp = inp.tile([P, nb2], f16, tag="hp", name="hp")

        def sc(tag, w):
            return scr.tile([P, cwmax], f16, tag=tag,
                            name="%s_%d" % (tag, sc.i))[:, :w]

        def mm_supertiles(t0, cw, M, W, vr, oht, out_d, tag):
            nq0, nq1 = t0 // 4, (t0 + cw) // 4
            pend = []
            for st0 in range(nq0, nq1, SQ):
                st1 = min(st0 + SQ, nq1)
                wcols = (st1 - st0) * M
                ps = psp.tile([P, SQ * M], f32, tag="ps" + tag, space="PSUM",
                              name="ps%s_%d" % (tag, st0))
                for q in range(st0, st1):
                    for g in range(4):
                        b = q * 4 + g
                        nc.tensor.matmul(
                            out=ps[32 * g:32 * g + W,
                                   (q - st0) * M:(q - st0 + 1) * M],
                            lhsT=vr[:, :, b - t0],
                            rhs=oht[:, (b - t0) * M:(b - t0 + 1) * M],
                            start=True, stop=True, skip_group_check=True,
                            tile_position=(0, 32 * g))

                def emit_copy(ps=ps, st0=st0, wcols=wcols):
                    cpt = outp.tile([P, SQ * M], f16, tag="cp" + tag,
                                    name="cp%s_%d" % (tag, st0))
                    if (st0 // SQ) % 2 == 0:
                        A.activation(out=cpt[:, :wcols], in_=ps[:, :wcols],
                                     func=AF.Copy)
                    else:
                        V.tensor_scalar(out=cpt[:, :wcols], in0=ps[:, :wcols],
                                        scalar1=1.0, scalar2=None, op0=OP.mult)
                    nc.sync.dma_start(out=out_d[:, st0 * M:st0 * M + wcols],
                                      in_=cpt[:, :wcols])
                pend.append(emit_copy)
            return pend

        def g2_chunk(ci):
            t0, cw = ch2[ci]
            sl = slice(t0, t0 + cw)
            oht = oh_fetch("2", ci)
            q = sc("q2", cw)
            A.activation(out=q, in_=d2[:, sl], func=AF.Square)
            if ci == 0:
                V.tensor_scalar(out=hp[:], in0=h[:], scalar1=1.0,
                                scalar2=None, op0=OP.add)
                A.activation(out=ub[:], in_=dba, func=AF.Sin,
                             bias=const(math.pi / 2),
                             scale=const(-math.pi / 6))
                A.activation(out=ucp[:], in_=dca, func=AF.Sin,
                             bias=const(math.pi / 2),
                             scale=const(-math.pi / 6))
            v8 = vchk.tile([P, W2 * cwmax], f16, tag="v8", name="v8_%d" % ci)
            v8r = v8[:].rearrange("p (w t) -> p w t", t=cwmax)
            ge = {}
            for k in (0, 1, 3, 4, 5):
                ge[k] = sc("ge%d" % k, cw)
                A.activation(out=ge[k], in_=q, func=AF.Exp,
                             scale=const(-float(g2e[k])))
            for k, ksrc in ((2, 1), (6, 5), (7, 6)):
                ge[k] = sc("ge%d" % k, cw)
                A.activation(out=ge[k], in_=ge[ksrc], func=AF.Square)
            for k in range(8):
                V.tensor_tensor(out=v8r[:, k, :cw], in0=hp[:, sl], in1=ge[k],
                                op=OP.mult)
            return mm_supertiles(t0, cw, M2, W2, v8r, oht[:], out2_d[:], "2")

        def g4_chunk(ci):
            t0, cw = ch4[ci]
            sl = slice(t0, t0 + cw)
            oht = oh_fetch("4", ci)
            b2 = sc("b2", cw)
            A.activation(out=b2, in_=dba[:, sl], func=AF.Square)
            c2 = sc("c2", cw)
            A.activation(out=c2, in_=dca[:, sl], func=AF.Square)
            c2n = sc("c2n", cw)
            A.activation(out=c2n, in_=cph[:, sl], func=AF.Square)
            bc = sc("bc", cw)
            G.tensor_tensor(out=bc, in0=dba[:, sl], in1=dca[:, sl], op=OP.mult)
            c3 = sc("c3", cw)
            G.tensor_tensor(out=c3, in0=c2n, in1=cph[:, sl], op=OP.mult)
            c4 = sc("c4", cw)
            G.tensor_tensor(out=c4, in0=c2n, in1=c2n, op=OP.mult)
            t4 = sc("t4", cw)
            V.tensor_tensor(out=t4, in0=b2, in1=c2, op=OP.add)
            bcc = sc("bcc", cw)
            V.tensor_tensor(out=bcc, in0=bc, in1=cph[:, sl], op=OP.mult)
            u = sc("u", cw)
            V.tensor_tensor(out=u, in0=t4, in1=bcc, op=OP.subtract)
            r2 = sc("r2", cw)
            V.tensor_tensor(out=r2, in0=u, in1=bcc, op=OP.subtract)
            yc = sc("yc", cw)
            V.tensor_scalar(out=yc, in0=r2, scalar1=1.0 / 36.0, scalar2=1.0,
                            op0=OP.mult, op1=OP.min)
            pv = sc("pv", cw)
            V.tensor_scalar(out=pv, in0=yc, scalar1=co[5], scalar2=None,
                            op0=OP.mult)
            for k in (4, 3, 2, 1):
                V.scalar_tensor_tensor(out=pv, in0=pv, scalar=co[k], in1=yc,
                                       op0=OP.add, op1=OP.mult)
            cut = sc("cut", cw)
            V.scalar_tensor_tensor(out=cut, in0=pv, scalar=co[0],
                                   in1=uu[:, sl], op0=OP.add, op1=OP.mult)
            e1 = sc("e1", cw)
            V.tensor_scalar(out=e1, in0=u, scalar1=-2.0 * float(etas[0]),
                            scalar2=1.0, op0=OP.mult, op1=OP.add)
            e2 = sc("e2", cw)
            A.activation(out=e2, in_=u, func=AF.Exp,
                         scale=const(-2.0 * float(etas[1])))
            e3 = sc("e3", cw)
            A.activation(out=e3, in_=u, func=AF.Exp,
                         scale=const(-2.0 * float(etas[2])))
            v15 = vchk.tile([P, W4 * cwmax], f16, tag="v15", name="v15_%d" % ci)
            v15r = v15[:].rearrange("p (w t) -> p w t", t=cwmax)
            pows = [None, cph[:, sl], c2n, c3, c4]
            for i, e in enumerate((e1, e2, e3)):
                av = v15r[:, i * 5, :cw]
                V.tensor_tensor(out=av, in0=e, in1=cut, op=OP.mult)
                for k in range(1, 5):
                    eng = G if (i, k) in ((0, 2), (1, 3), (2, 4), (2, 2)) else V
                    eng.tensor_tensor(out=v15r[:, i * 5 + k, :cw],
                                      in0=av, in1=pows[k], op=OP.mult)
            return mm_supertiles(t0, cw, M4, W4, v15r, oht[:], out4_d[:], "4")

        pend = []
        for ci in range(max(len(ch2), len(ch4))):
            sc.i = ci
            newpend = []
            if ci < len(ch2):
                newpend += g2_chunk(ci)
            if ci == 0:
                uu_emit()
            if ci < len(ch4):
                newpend += g4_chunk(ci)
            for fn in pend:
                fn()
            pend = newpend
        for fn in pend:
            fn()
    nc.finalize()
    return nc


# --------------------------------------------------------------------------
# entry point
# --------------------------------------------------------------------------

def _run(inputs, trace=False):
    import ml_dtypes
    from concourse.bass_utils import run_bass_kernel_spmd

    plan = _plan(inputs)
    consts = {k: plan[k] for k in ("etas", "g2_etas")}
    nc = _build_nc(plan["nb4"], plan["nb2"], consts)

    in_maps = []
    for c in range(N_CORES):
        in_maps.append(dict(
            dba=plan["dba"][c], dca=plan["dca"][c], cph=plan["cph"][c],
            d2=plan["d2"][c],
            oh4=plan["oh4"][c].view(ml_dtypes.float8_e4m3fn),
            oh2=plan["oh2"][c].view(ml_dtypes.float8_e4m3fn)))
    res = run_bass_kernel_spmd(nc, in_maps, core_ids=list(range(N_CORES)),
                               trace=trace)
    out = _assemble([r["out4"] for r in res.results],
                    [r["out2"] for r in res.results], plan)
    return out, res


def kernel(**inputs):
    return _run(inputs)[0]


# revision 19
# speedup vs baseline: 1.0168x; 1.0168x over previous
"""ACSF descriptor kernel for 8 Trainium2 NeuronCores (Bass/Tile), v4.

Scheme
------
* Output atoms sharded across 8 cores (6250 each).  Host does integer
  topology routing plus standard neighbor-list cutoff pruning: G4
  triplets with D_ba >= rc, D_ca >= rc or R_bc >= rc contribute exactly
  zero (the reference multiplies them by 0) and are dropped (37%); G2
  edges with D > 5.8 contribute < 3e-3 of one output unit (fc(5.8) =
  2.7e-3) and are dropped (tolerance is 2e-2; measured error ~1e-3).
* Atoms are packed BLK per block (G4: 10, G2: 7) so each block is
  exactly ONE 128-item tile; ONE matmul per block scatters 15 (G4
  "moment" planes av_i cos^k) / 8 (G2) values into [W, 3*BLK] PSUM via
  a tiny fp8 one-hot.  The PE retires (LDWEIGHTS, MATMUL) pairs at a
  fixed ~34ns regardless of width, so minimizing matmul count (= block
  count) is everything: 628 + 896 MMs per core.
* Host reconstructs the (lambda, zeta) G4 columns from the moments with
  binomial combinations (exact), and scales by 2^(1-z)/8 and 0.5.
* No activation-table thrash: 3 input cosines (Sin) run up front, then
  Exp/Square only (2 table loads).  fc(R_bc) is a degree-5 polynomial
  in R^2/36 (pinned to 0 at rc) on DVE; dba/dca are host-clamped to 6.
  3 of 8 G2 exps are ACT squares of others (eta ratios are powers of 2).
* fp16 streams/intermediates/outputs; values chunked (geometrically
  growing chunks) so the matmul stream starts early; PSUM->SBUF copies
  alternate between Scalar and Vector and are emitted one chunk late to
  avoid head-of-line blocking.
"""

import math
from contextlib import ExitStack

import numpy as np

P = 128
N_ATOMS = 50000
N_CORES = 8
APC = N_ATOMS // N_CORES          # 6250 atoms per core
BLK4, W4 = 10, 15                 # G4: atoms/block, value planes
M4 = 3 * BLK4
BLK2, W2 = 7, 8                   # G2
M2 = 2 * BLK2
SQ = 16                           # quads per supertile (64 blocks)
RC = 6.0
G2_CUT = 5.8                      # G2 prune radius (fc(5.8) = 2.7e-3)


def _fit_poly():
    y = np.linspace(0.0, 1.0, 4001)
    V = np.vander(y, 6, increasing=True)
    c, *_ = np.linalg.lstsq(V, 1.0 + np.cos(np.pi * np.sqrt(y)), rcond=None)
    c[0] -= c.sum()
    return c.astype(np.float64)

POLY = _fit_poly()


# --------------------------------------------------------------------------
# host-side planning (integer topology work + cutoff pruning only)
# --------------------------------------------------------------------------

def _lpt_pack(cnts, nb, blkcap):
    """Place atoms (desc by count) into the least-loaded block that still
    has atom and item capacity.  Returns None if nb blocks don't suffice."""
    import heapq
    order = np.argsort(-cnts, kind="stable")
    heap = [(0, 0, b) for b in range(nb)]
    heapq.heapify(heap)
    items = np.zeros(nb, np.int64)
    nat = np.zeros(nb, np.int64)
    blk = np.empty(APC, np.int32)
    alo = np.empty(APC, np.int32)
    for a in order:
        c = int(cnts[a])
        stash = []
        ok = False
        while heap:
            it, na, b = heapq.heappop(heap)
            if it != items[b] or na != nat[b]:
                continue
            if na >= blkcap or it + c > P:
                stash.append((it, na, b))
                continue
            blk[a] = b
            alo[a] = na
            items[b] += c
            nat[b] += 1
            if nat[b] < blkcap:
                heapq.heappush(heap, (items[b], nat[b], b))
            ok = True
            break
        for s in stash:
            heapq.heappush(heap, s)
        if not ok:
            return None
    return blk, alo


def _balance(cnt_all, blkcap):
    nb = -(-max(-(-APC // blkcap), int(-(-cnt_all.sum(1).max() // P))) // 4) * 4
    while True:
        packs = []
        for c in range(N_CORES):
            r = _lpt_pack(cnt_all[c], nb, blkcap)
            if r is None:
                break
            packs.append(r)
        if len(packs) == N_CORES:
            return (np.stack([p[0] for p in packs]),
                    np.stack([p[1] for p in packs]), nb)
        nb += 8


def _pack(n_global, slot, feats, defaults, blk_of, aloc_of, nblk, M, nsl):
    """Route items into (core, block, partition) slots; emit fp16 value
    streams [C, 128, nblk] and the fp8 one-hot [C, 128, nblk*M]."""
    core = n_global // APC
    al = n_global % APC
    b = blk_of[core, al].astype(np.int64)
    nv = aloc_of[core, al].astype(np.int64) * nsl + slot
    key = core * nblk + b
    cnt = np.bincount(key, minlength=N_CORES * nblk)
    assert cnt.max() <= P, cnt.max()
    order = np.argsort(key, kind="stable")
    starts = np.zeros(N_CORES * nblk, np.int64)
    np.cumsum(cnt[:-1], out=starts[1:])
    rank = np.arange(len(key), dtype=np.int64) - np.repeat(starts, cnt)
    ko = key[order]
    co, bo, po = ko // nblk, ko % nblk, rank
    arrs = []
    for f, d in zip(feats, defaults):
        a = np.full((N_CORES, P, nblk), d, np.float16)
        a[co, po, bo] = f[order].astype(np.float16)
        arrs.append(a)
    oh = np.zeros((N_CORES, P, nblk * M), np.uint8)
    oh[co, po, bo * M + nv[order]] = 0x38          # 1.0 in fp8e4m3
    return arrs, oh


def _plan(inputs):
    an = np.asarray(inputs["atomic_numbers"])
    ei = np.asarray(inputs["edge_index"])
    D_st = np.asarray(inputs["D_st"], np.float32)
    ba = np.asarray(inputs["id3_ba"])
    ca = np.asarray(inputs["id3_ca"])
    cph = np.asarray(inputs["cos_phi"], np.float32)
    imap = np.asarray(inputs["idx_mapping"])
    imap2 = np.asarray(inputs["idx_mapping_g2"])
    src, dst = ei[0], ei[1]

    # ---- G4: integer mask + cutoff prune + destination/slot computation
    keep = ba > ca
    ba = ba[keep]; ca = ca[keep]; ch = cph[keep]
    db = D_st[ba]; dc = D_st[ca]
    alive = (db < RC) & (dc < RC) & (db * db + dc * dc - 2 * db * dc * ch
                                     < RC * RC)
    ba = ba[alive]; ca = ca[alive]
    n4 = dst[ca]
    p4 = imap[an[dst[ca]], an[src[ba]], an[src[ca]]]
    cnt4 = np.bincount(n4, minlength=N_ATOMS).reshape(N_CORES, APC)
    blk4, aloc4, nb4 = _balance(cnt4, BLK4)
    g4_arrs, oh4 = _pack(
        n4, p4, [db[alive], dc[alive], ch[alive]],
        [1.0, 1.0, 0.0], blk4, aloc4, nb4, M4, 3)

    # ---- G2: prune negligible-contribution edges
    alive2 = D_st <= G2_CUT
    n2 = dst[alive2]
    s2 = imap2[an[dst[alive2]], an[src[alive2]]]
    cnt2 = np.bincount(n2, minlength=N_ATOMS).reshape(N_CORES, APC)
    blk2, aloc2, nb2 = _balance(cnt2, BLK2)
    g2_arrs, oh2 = _pack(n2, s2, [D_st[alive2]], [1.0], blk2, aloc2,
                         nb2, M2, 2)

    g2_etas = np.asarray(inputs["G2_params"], np.float64)[0, 0]        # [8]
    etas = np.asarray(inputs["G4_etas"], np.float64)[0, 0, 0]          # [3]
    zetas = np.asarray(inputs["G4_zetas"], np.float64)[0, 0, 0]        # [3]
    lmdas = np.asarray(inputs["G4_lmdas"], np.float64)[0, 0, 0]        # [2]
    assert np.allclose(zetas, [1.0, 2.0, 4.0]), zetas
    assert np.allclose(sorted(lmdas), [-1.0, 1.0]), lmdas
    # eta relations used to replace 3 of the 8 G2 exps with ACT squarings
    assert abs(g2_etas[2] - 2 * g2_etas[1]) < 1e-4 * g2_etas[2]
    assert abs(g2_etas[6] - 2 * g2_etas[5]) < 1e-4 * g2_etas[6]
    assert abs(g2_etas[7] - 2 * g2_etas[6]) < 1e-4 * g2_etas[7]
    assert etas[0] * 220.0 < 0.1        # linearized exp(-eta0*s) stays accurate

    return dict(
        dba=g4_arrs[0], dca=g4_arrs[1], cph=g4_arrs[2], oh4=oh4,
        d2=g2_arrs[0], oh2=oh2,
        blk4=blk4, aloc4=aloc4, nb4=nb4,
        blk2=blk2, aloc2=aloc2, nb2=nb2,
        g2_etas=g2_etas, etas=etas, zetas=zetas, lmdas=lmdas,
    )


def _combo_matrix(zetas, lmdas):
    B = np.zeros((6, 5), np.float64)
    for li, l in enumerate(lmdas):
        for zi, z in enumerate(zetas):
            cz = 0.125 * 2.0 ** (1.0 - z)
            for k in range(int(z) + 1):
                B[li * 3 + zi, k] = math.comb(int(z), k) * (l ** k) * cz
    return B


def _assemble(o4s, o2s, plan):
    nb4, nb2 = plan["nb4"], plan["nb2"]
    B = _combo_matrix(plan["zetas"], plan["lmdas"])
    full = np.empty((N_ATOMS, 70), np.float32)
    for c in range(N_CORES):
        V4 = np.asarray(o4s[c], np.float32).reshape(4, 32, nb4 // 4, BLK4, 3)
        A4 = V4[:, :W4].transpose(2, 0, 3, 1, 4).reshape(nb4 * BLK4, W4, 3)
        r4 = plan["blk4"][c].astype(np.int64) * BLK4 + plan["aloc4"][c]
        Mm = A4[r4].reshape(APC, 3, 5, 3)
        g4 = np.einsum('aiks,wk->aiws', Mm, B).reshape(APC, 54)
        V2 = np.asarray(o2s[c], np.float32).reshape(4, 32, nb2 // 4, BLK2, 2)
        A2 = V2[:, :W2].transpose(2, 0, 3, 1, 4).reshape(nb2 * BLK2, W2, 2)
        r2 = plan["blk2"][c].astype(np.int64) * BLK2 + plan["aloc2"][c]
        g2 = (A2[r2] * 0.5).reshape(APC, 16)
        full[c * APC:(c + 1) * APC, :16] = g2
        full[c * APC:(c + 1) * APC, 16:] = g4
    return full


def _chunks(nb):
    """Geometrically growing chunk plan aligned to supertiles (64 blocks)."""
    out = []
    t0 = 0
    for w in (64, 96, 128, 192, 256, 256, 256, 256):
        if t0 >= nb:
            break
        cw = min(w, nb - t0)
        out.append((t0, cw))
        t0 += cw
    while t0 < nb:
        out.append((t0, min(256, nb - t0)))
        t0 += min(256, nb - t0)
    return out


# --------------------------------------------------------------------------
# numpy simulation of the device program (for host-side validation)
# --------------------------------------------------------------------------

def _simulate_core(plan, c):
    f16 = np.float16
    e1c, e2c, e3c = plan["etas"]
    g2e = plan["g2_etas"]
    dba = plan["dba"][c]; dca = plan["dca"][c]; cph = plan["cph"][c]
    d2 = plan["d2"][c]
    co = POLY

    def act(x, f):
        return f(x.astype(np.float32)).astype(f16)

    b2 = act(dba, np.square); c2 = act(dca, np.square)
    c2n = act(cph, np.square)
    bc = (dba * dca).astype(f16)
    c3 = (c2n * cph).astype(f16)
    c4 = (c2n * c2n).astype(f16)
    t4 = (b2 + c2).astype(f16)
    bcc = (bc * cph).astype(f16)
    u = (t4 - bcc).astype(f16)                      # s = 2u
    r2 = (u - bcc).astype(f16)
    yc = np.minimum((r2 * f16(1 / 36.)).astype(f16), f16(1.0))
    v = (yc * f16(co[5])).astype(f16)
    for k in (4, 3, 2, 1):
        v = ((v + f16(co[k])) * yc).astype(f16)
    ub = act(dba, lambda x: np.sin(np.pi / 2 - np.pi / 6 * x))
    uc = act(dca, lambda x: np.sin(np.pi / 2 - np.pi / 6 * x))
    uu = ((ub + f16(1.0)) * (uc + f16(1.0)).astype(f16)).astype(f16)
    cut = ((v + f16(co[0])) * uu).astype(f16)
    e1 = (u * f16(-2 * e1c) + f16(1.0)).astype(f16)
    e2 = act(u, lambda x: np.exp(-2 * e2c * x))
    e3 = act(u, lambda x: np.exp(-2 * e3c * x))
    pows = [None, cph, c2n, c3, c4]
    v15 = np.empty((P, W4, plan["nb4"]), f16)
    for i, e in enumerate((e1, e2, e3)):
        av = (e * cut).astype(f16)
        v15[:, i * 5] = av
        for k in range(1, 5):
            v15[:, i * 5 + k] = (av * pows[k]).astype(f16)
    oh4 = (plan["oh4"][c] == 0x38).reshape(P, plan["nb4"], M4)
    o4 = np.zeros((P, plan["nb4"] // 4 * M4), np.float32)
    for b in range(plan["nb4"]):
        q, g = b // 4, b % 4
        o4[32 * g:32 * g + W4, q * M4:(q + 1) * M4] = (
            v15[:, :, b].astype(np.float32).T @ oh4[:, b].astype(np.float32))
    q2 = act(d2, np.square)
    h = act(d2, lambda x: np.sin(np.pi / 2 - np.pi / 6 * x))
    hp = (h + f16(1.0)).astype(f16)
    ge = {}
    for k in (0, 1, 3, 4, 5):
        ge[k] = act(q2, lambda x, kk=k: np.exp(-g2e[kk] * x))
    ge[2] = act(ge[1], np.square)
    ge[6] = act(ge[5], np.square)
    ge[7] = act(ge[6], np.square)
    v8 = np.empty((P, W2, plan["nb2"]), f16)
    for k in range(8):
        v8[:, k] = (hp * ge[k]).astype(f16)
    oh2 = (plan["oh2"][c] == 0x38).reshape(P, plan["nb2"], M2)
    o2 = np.zeros((P, plan["nb2"] // 4 * M2), np.float32)
    for b in range(plan["nb2"]):
        q, g = b // 4, b % 4
        o2[32 * g:32 * g + W2, q * M2:(q + 1) * M2] = (
            v8[:, :, b].astype(np.float32).T @ oh2[:, b].astype(np.float32))
    return o4.astype(f16), o2.astype(f16)


def simulate(inputs):
    plan = _plan(inputs)
    outs = [_simulate_core(plan, c) for c in range(N_CORES)]
    return _assemble([o[0] for o in outs], [o[1] for o in outs], plan)


# --------------------------------------------------------------------------
# Bass/Tile device kernel
# --------------------------------------------------------------------------

def _build_nc(nb4, nb2, consts):
    import concourse.bacc as bacc
    import concourse.tile as tile
    from concourse import mybir

    f32 = mybir.dt.float32
    f16 = mybir.dt.float16
    f8 = mybir.dt.float8e4
    AF = mybir.ActivationFunctionType
    OP = mybir.AluOpType
    etas, g2e = consts["etas"], consts["g2_etas"]
    co = [float(x) for x in POLY]

    nc = bacc.Bacc(None, target_bir_lowering=False)
    dba_d = nc.dram_tensor("dba", [P, nb4], f16, kind="ExternalInput")
    dca_d = nc.dram_tensor("dca", [P, nb4], f16, kind="ExternalInput")
    cph_d = nc.dram_tensor("cph", [P, nb4], f16, kind="ExternalInput")
    d2_d = nc.dram_tensor("d2", [P, nb2], f16, kind="ExternalInput")
    oh4_d = nc.dram_tensor("oh4", [P, nb4 * M4], f8, kind="ExternalInput")
    oh2_d = nc.dram_tensor("oh2", [P, nb2 * M2], f8, kind="ExternalInput")
    out4_d = nc.dram_tensor("out4", [P, nb4 // 4 * M4], f16,
                            kind="ExternalOutput")
    out2_d = nc.dram_tensor("out2", [P, nb2 // 4 * M2], f16,
                            kind="ExternalOutput")

    ch4 = _chunks(nb4)
    ch2 = _chunks(nb2)
    cwmax = 256

    with tile.TileContext(nc) as tc, ExitStack() as ctx:
        inp = ctx.enter_context(tc.tile_pool(name="inp", bufs=1))
        scr = ctx.enter_context(tc.tile_pool(name="scr", bufs=3))
        vchk = ctx.enter_context(tc.tile_pool(name="vchk", bufs=3))
        ohp = ctx.enter_context(tc.tile_pool(name="ohp", bufs=4))
        outp = ctx.enter_context(tc.tile_pool(name="outp", bufs=3))
        psp = ctx.enter_context(tc.tile_pool(name="psum", bufs=4, space="PSUM"))

        V, A, G = nc.vector, nc.scalar, nc.gpsimd

        consts_sb = {}

        def const(v):
            v = float(v)
            if v not in consts_sb:
                tl = inp.tile([P, 1], f32, tag="const%r" % v,
                              name="c%d" % len(consts_sb))
                nc.vector.memset(tl[:], v)
                consts_sb[v] = tl[:]
            return consts_sb[v]

        oh_tiles = {}

        def oh_fetch(fam, ci):
            key = (fam, ci)
            if key not in oh_tiles:
                M, nb, dd, cl = ((M2, nb2, oh2_d, ch2) if fam == "2"
                                 else (M4, nb4, oh4_d, ch4))
                t0, cw = cl[ci]
                t = ohp.tile([P, cwmax * M], f8, tag="oh" + fam,
                             name="oh%s_%d" % (fam, ci))
                nc.sync.dma_start(out=t[:, :cw * M],
                                  in_=dd[:, t0 * M:(t0 + cw) * M])
                oh_tiles[key] = t
            return oh_tiles[key]

        # dummy activations: hoist the two ACT table loads ahead of the
        # input DMAs (table loads have no data dependency)
        dum = inp.tile([P, 1], f16, tag="dum", name="dum")
        A.activation(out=dum[:], in_=const(0.0), func=AF.Sin)

        # ---- input DMAs: G2's dependencies first so its MMs start early ----
        sb = {}
        sb["d2"] = inp.tile([P, nb2], f16, tag="d2", name="sb_d2")
        nc.sync.dma_start(out=sb["d2"][:], in_=d2_d[:])
        oh_fetch("2", 0)
        for nm, dd in (("dba", dba_d), ("dca", dca_d), ("cph", cph_d)):
            sb[nm] = inp.tile([P, nb4], f16, tag=nm, name="sb_" + nm)
            nc.sync.dma_start(out=sb[nm][:], in_=dd[:])
        oh_fetch("4", 0)
        dba, dca, cph, d2 = (sb[k][:] for k in ("dba", "dca", "cph", "d2"))

        # ---- the only Sin-table users, then Exp/Square forever ----
        h = inp.tile([P, nb2], f16, tag="h", name="h")
        ub = inp.tile([P, nb4], f16, tag="ub", name="ub")
        ucp = inp.tile([P, nb4], f16, tag="ucp", name="ucp")
        uu = inp.tile([P, nb4], f16, tag="uu", name="uu")
        with tc.high_priority():
            A.activation(out=h[:], in_=d2, func=AF.Sin,
                         bias=const(math.pi / 2), scale=const(-math.pi / 6))
        def uu_emit():
            A.activation(out=ub[:], in_=dba, func=AF.Sin,
                         bias=const(math.pi / 2), scale=const(-math.pi / 6))
            A.activation(out=ucp[:], in_=dca, func=AF.Sin,
                         bias=const(math.pi / 2), scale=const(-math.pi / 6))
            V.tensor_scalar(out=ucp[:], in0=ucp[:], scalar1=1.0, scalar2=None,
                            op0=OP.add)
            V.scalar_tensor_tensor(out=uu[:], in0=ub[:], scalar=1.0,
                                   in1=ucp[:], op0=OP.add, op1=OP.mult)

        hp = inp.tile([P, nb2], f16, tag="hp", name="hp")

        def sc(tag, w):
            return scr.tile([P, cwmax], f16, tag=tag,
                            name="%s_%d" % (tag, sc.i))[:, :w]

        def mm_supertiles(t0, cw, M, W, vr, oht, out_d, tag):
            nq0, nq1 = t0 // 4, (t0 + cw) // 4
            pend = []
            for st0 in range(nq0, nq1, SQ):
                st1 = min(st0 + SQ, nq1)
                wcols = (st1 - st0) * M
                ps = psp.tile([P, SQ * M], f32, tag="ps" + tag, space="PSUM",
                              name="ps%s_%d" % (tag, st0))
                for q in range(st0, st1):
                    for g in range(4):
                        b = q * 4 + g
                        nc.tensor.matmul(
                            out=ps[32 * g:32 * g + W,
                                   (q - st0) * M:(q - st0 + 1) * M],
                            lhsT=vr[:, :, b - t0],
                            rhs=oht[:, (b - t0) * M:(b - t0 + 1) * M],
                            start=True, stop=True, skip_group_check=True,
                            tile_position=(0, 32 * g))

                def emit_copy(ps=ps, st0=st0, wcols=wcols):
                    cpt = outp.tile([P, SQ * M], f16, tag="cp" + tag,
                                    name="cp%s_%d" % (tag, st0))
                    if (st0 // SQ) % 2 == 0:
                        A.activation(out=cpt[:, :wcols], in_=ps[:, :wcols],
                                     func=AF.Copy)
                    else:
                        V.tensor_scalar(out=cpt[:, :wcols], in0=ps[:, :wcols],
                                        scalar1=1.0, scalar2=None, op0=OP.mult)
                    nc.sync.dma_start(out=out_d[:, st0 * M:st0 * M + wcols],
                                      in_=cpt[:, :wcols])
                pend.append(emit_copy)
            return pend

        def g2_chunk(ci):
            t0, cw = ch2[ci]
            sl = slice(t0, t0 + cw)
            oht = oh_fetch("2", ci)
            q = sc("q2", cw)
            A.activation(out=q, in_=d2[:, sl], func=AF.Square)
            if ci == 0:
                V.tensor_scalar(out=hp[:], in0=h[:], scalar1=1.0,
                                scalar2=None, op0=OP.add)
            v8 = vchk.tile([P, W2 * cwmax], f16, tag="v8", name="v8_%d" % ci)
            v8r = v8[:].rearrange("p (w t) -> p w t", t=cwmax)
            ge = {}
            for k in (0, 1, 3, 4, 5):
                ge[k] = sc("ge%d" % k, cw)
                A.activation(out=ge[k], in_=q, func=AF.Exp,
                             scale=const(-float(g2e[k])))
            for k, ksrc in ((2, 1), (6, 5), (7, 6)):
                ge[k] = sc("ge%d" % k, cw)
                V.tensor_tensor(out=ge[k], in0=ge[ksrc], in1=ge[ksrc],
                                op=OP.mult)
            for k in range(8):
                V.tensor_tensor(out=v8r[:, k, :cw], in0=hp[:, sl], in1=ge[k],
                                op=OP.mult)
            return mm_supertiles(t0, cw, M2, W2, v8r, oht[:], out2_d[:], "2")

        def g4_chunk(ci):
            t0, cw = ch4[ci]
            sl = slice(t0, t0 + cw)
            oht = oh_fetch("4", ci)
            b2 = sc("b2", cw)
            A.activation(out=b2, in_=dba[:, sl], func=AF.Square)
            c2 = sc("c2", cw)
            A.activation(out=c2, in_=dca[:, sl], func=AF.Square)
            c2n = sc("c2n", cw)
            A.activation(out=c2n, in_=cph[:, sl], func=AF.Square)
            bc = sc("bc", cw)
            G.tensor_tensor(out=bc, in0=dba[:, sl], in1=dca[:, sl], op=OP.mult)
            c3 = sc("c3", cw)
            G.tensor_tensor(out=c3, in0=c2n, in1=cph[:, sl], op=OP.mult)
            c4 = sc("c4", cw)
            G.tensor_tensor(out=c4, in0=c2n, in1=c2n, op=OP.mult)
            t4 = sc("t4", cw)
            V.tensor_tensor(out=t4, in0=b2, in1=c2, op=OP.add)
            bcc = sc("bcc", cw)
            V.tensor_tensor(out=bcc, in0=bc, in1=cph[:, sl], op=OP.mult)
            u = sc("u", cw)
            V.tensor_tensor(out=u, in0=t4, in1=bcc, op=OP.subtract)
            r2 = sc("r2", cw)
            V.tensor_tensor(out=r2, in0=u, in1=bcc, op=OP.subtract)
            yc = sc("yc", cw)
            V.tensor_scalar(out=yc, in0=r2, scalar1=1.0 / 36.0, scalar2=1.0,
                            op0=OP.mult, op1=OP.min)
            pv = sc("pv", cw)
            V.tensor_scalar(out=pv, in0=yc, scalar1=co[5], scalar2=None,
                            op0=OP.mult)
            for k in (4, 3, 2, 1):
                V.scalar_tensor_tensor(out=pv, in0=pv, scalar=co[k], in1=yc,
                                       op0=OP.add, op1=OP.mult)
            cut = sc("cut", cw)
            V.scalar_tensor_tensor(out=cut, in0=pv, scalar=co[0],
                                   in1=uu[:, sl], op0=OP.add, op1=OP.mult)
            e1 = sc("e1", cw)
            V.tensor_scalar(out=e1, in0=u, scalar1=-2.0 * float(etas[0]),
                            scalar2=1.0, op0=OP.mult, op1=OP.add)
            e2 = sc("e2", cw)
            A.activation(out=e2, in_=u, func=AF.Exp,
                         scale=const(-2.0 * float(etas[1])))
            e3 = sc("e3", cw)
            A.activation(out=e3, in_=u, func=AF.Exp,
                         scale=const(-2.0 * float(etas[2])))
            v15 = vchk.tile([P, W4 * cwmax], f16, tag="v15", name="v15_%d" % ci)
            v15r = v15[:].rearrange("p (w t) -> p w t", t=cwmax)
            pows = [None, cph[:, sl], c2n, c3, c4]
            for i, e in enumerate((e1, e2, e3)):
                av = v15r[:, i * 5, :cw]
                V.tensor_tensor(out=av, in0=e, in1=cut, op=OP.mult)
                for k in range(1, 5):
                    eng = G if (i, k) in ((0, 2), (1, 3), (2, 4), (2, 2)) else V
                    eng.tensor_tensor(out=v15r[:, i * 5 + k, :cw],
                                      in0=av, in1=pows[k], op=OP.mult)
            return mm_supertiles(t0, cw, M4, W4, v15r, oht[:], out4_d[:], "4")

        pend = []
        for ci in range(max(len(ch2), len(ch4))):
            sc.i = ci
            newpend = []
            if ci < len(ch2):
                newpend += g2_chunk(ci)
            if ci == 0:
                uu_emit()
            if ci < len(ch4):
                newpend += g4_chunk(ci)
            if ci + 2 < len(ch2):
                oh_fetch("2", ci + 2)
            if ci + 2 < len(ch4):
                oh_fetch("4", ci + 2)
            for fn in pend:
                fn()
            pend = newpend
        for fn in pend:
            fn()
    nc.finalize()
    return nc


# --------------------------------------------------------------------------
# entry point
# --------------------------------------------------------------------------

def _run(inputs, trace=False):
    import ml_dtypes
    from concourse.bass_utils import run_bass_kernel_spmd

    plan = _plan(inputs)
    consts = {k: plan[k] for k in ("etas", "g2_etas")}
    nc = _build_nc(plan["nb4"], plan["nb2"], consts)

    in_maps = []
    for c in range(N_CORES):
        in_maps.append(dict(
            dba=plan["dba"][c], dca=plan["dca"][c], cph=plan["cph"][c],
            d2=plan["d2"][c],
            oh4=plan["oh4"][c].view(ml_dtypes.float8_e4m3fn),
            oh2=plan["oh2"][c].view(ml_dtypes.float8_e4m3fn)))
    res = run_bass_kernel_spmd(nc, in_maps, core_ids=list(range(N_CORES)),
                               trace=trace)
    out = _assemble([r["out4"] for r in res.results],
                    [r["out2"] for r in res.results], plan)
    return out, res


def kernel(**inputs):
    return _run(inputs)[0]
